# revision 1
# baseline (speedup 1.0000x reference)
"""Trainium2 Bass kernel for nn_MultiHeadAttention (decode-style, q_len=1).

Data-parallel over batch: 64 batches -> 8 cores x 8 batches.

Key algebraic restructuring (exact, exploits q_len == 1):
  scores[b,h,s] = (q Wq + bq)_h . (k Wk + bk)_h
                = k[b,s,:] . R_b[:,h] + const(b,h)        # const drops in softmax
     where R_b[d,h] = sum_{d'} Wk[d, h*64+d'] qh[b, h*64+d']
  out_concat[b,hd] = (sum_s p[b,h,s] v[b,s,:]) @ Wv[:,hd] + bv[hd]
so the big K/V projections (2 x 275 GFLOP) are never computed; instead
k and v are contracted directly (2 x 4.3 GFLOP) and the kernel becomes
HBM-bound on streaming k,v (128 MiB/core).

Matmuls on the big streams use float32r (TF32-like, ~1e-4 rel err).
k is transposed on-chip via PE transposes; rounding to f32r rides the
mandatory PSUM->SBUF copies.
"""

import numpy as np
from contextlib import ExitStack

import concourse.bass as bass
import concourse.tile as tile
from concourse import bacc, mybir
from concourse.bass_utils import run_bass_kernel_spmd

try:
    import axon_profile_shim
    axon_profile_shim.install()
except Exception:
    pass

N_CORES = 8
D = 1024
H = 16
DK = 64
F32 = mybir.dt.float32
F32R = mybir.dt.float32r
BF16 = mybir.dt.bfloat16
AX = mybir.AxisListType
ALU = mybir.AluOpType
ACTF = mybir.ActivationFunctionType


def _make_identity(nc, ap):
    nc.gpsimd.memset(ap, 0.0)
    nc.gpsimd.affine_select(
        out=ap, in_=ap, compare_op=ALU.not_equal, fill=1.0,
        base=0, pattern=[[-1, ap.shape[0]]], channel_multiplier=1,
    )


def build(BL=8, S=2048, n_cores=N_CORES):
    """Build + compile the per-core program. BL = local batches, S = seq len."""
    SC = S // 128          # 128-row s-subchunks
    SG = S // 512          # 512-row s-groups
    nc = bacc.Bacc("TRN2", target_bir_lowering=False, debug=False,
                   num_devices=n_cores)

    q_ext = nc.dram_tensor("q", [BL, D], F32, kind="ExternalInput").ap()
    k_ext = nc.dram_tensor("k", [BL * S, D], F32, kind="ExternalInput").ap()
    v_ext = nc.dram_tensor("v", [BL * S, D], F32, kind="ExternalInput").ap()
    Wq_ext = nc.dram_tensor("Wq", [D, D], F32, kind="ExternalInput").ap()
    Wk_ext = nc.dram_tensor("Wk", [D, D], F32, kind="ExternalInput").ap()
    Wv_ext = nc.dram_tensor("Wv", [D, D], F32, kind="ExternalInput").ap()
    Wo_ext = nc.dram_tensor("Wo", [D, D], F32, kind="ExternalInput").ap()
    bq_ext = nc.dram_tensor("bq", [D], F32, kind="ExternalInput").ap()
    bv_ext = nc.dram_tensor("bv", [D], F32, kind="ExternalInput").ap()
    bo_ext = nc.dram_tensor("bo", [D], F32, kind="ExternalInput").ap()
    y_ext = nc.dram_tensor("y", [BL, D], F32, kind="ExternalOutput").ap()

    with tile.TileContext(nc) as tc, ExitStack() as ctx:
        cpool = ctx.enter_context(tc.tile_pool(name="const", bufs=1))
        ident = cpool.tile([128, 128], F32)
        _make_identity(nc, ident[:])
        ident_bf = cpool.tile([128, 128], BF16)
        nc.vector.tensor_copy(ident_bf[:], ident[:])
        bv8 = cpool.tile([BL, D], F32)
        nc.sync.dma_start(bv8[:], bv_ext.unsqueeze(0).broadcast_to([BL, D]))
        bo8 = cpool.tile([BL, D], F32)
        nc.sync.dma_start(bo8[:], bo_ext.unsqueeze(0).broadcast_to([BL, D]))

        # persistent across whole kernel
        zeros32 = cpool.tile([128, 128], F32)
        nc.vector.memset(zeros32[:], 0.0)
        R_all = cpool.tile([128, 8, 16 * BL], BF16)
        UT_all = cpool.tile([128, 8, H, BL], F32)

        # ---------------- stream pools + k(0) prefetch ----------------
        kpool = ctx.enter_context(tc.tile_pool(name="kpool", bufs=5))
        ktpool = ctx.enter_context(tc.tile_pool(name="ktpool", bufs=3))
        vrpool = ctx.enter_context(tc.tile_pool(name="vrpool", bufs=4))
        epool = ctx.enter_context(tc.tile_pool(name="epool", bufs=2))
        etpool = ctx.enter_context(tc.tile_pool(name="etpool", bufs=2))
        upool = ctx.enter_context(tc.tile_pool(name="upool", bufs=2))

        def load_k(b):
            tiles = []
            for g in range(SG):
                kbf = kpool.tile([128, 4, D], BF16, tag="kbf", name="kbf")
                r0 = b * S + g * 512
                # stripe-4 load: partition p holds rows 4p..4p+3 (16KB
                # contiguous descriptors); the s-permutation e=j*128+p is
                # applied identically to k and v, and softmax/U are
                # order-invariant in s.
                nc.gpsimd.dma_start(
                    kbf[:], k_ext[r0:r0 + 512, :].rearrange("(p j) d -> p j d", p=128))
                tiles.append(kbf)
            return tiles

        def load_v(b):
            tiles = []
            for g in range(SG):
                vr = vrpool.tile([128, 4, D], BF16, tag="vr", name="vr")
                r0 = b * S + g * 512
                nc.gpsimd.dma_start(
                    vr[:], v_ext[r0:r0 + 512, :].rearrange("(p j) d -> p j d", p=128))
                tiles.append(vr)
            return tiles

        k_tiles = load_k(0)
        v_tiles0 = load_v(0)

        # ---------------- setup: qh^T, Wk^T, R ----------------
        with tc.tile_pool(name="wsetup", bufs=1) as wpool, \
             tc.tile_pool(name="wstage", bufs=2) as wstage, \
             tc.tile_pool(name="spsum", bufs=1, space="PSUM") as spsum:
            bq8 = wpool.tile([BL, D], F32)
            nc.sync.dma_start(bq8[:], bq_ext.unsqueeze(0).broadcast_to([BL, D]))
            Q = wpool.tile([BL, D], F32)
            nc.sync.dma_start(Q[:], q_ext[:])

            qtp = spsum.tile([128, 8 * BL], F32, tag="qtp")
            for i in range(8):
                nc.tensor.transpose(qtp[:, i * BL:(i + 1) * BL],
                                    Q[:, i * 128:(i + 1) * 128], ident[:BL, :BL])
            QT_sb = wpool.tile([128, 8 * BL], F32)
            nc.vector.tensor_copy(QT_sb[:], qtp[:])

            # qh = Q @ Wq + bq, with Wq streamed through a rotating stage
            qhp = spsum.tile([BL, D], F32, tag="qhp")
            for i in range(8):
                wq_st = wstage.tile([128, D], F32, tag="wq_st", name="wq_st")
                nc.sync.dma_start(wq_st[:], Wq_ext[i * 128:(i + 1) * 128, :])
                for n in range(2):
                    nc.tensor.matmul(qhp[:, n * 512:(n + 1) * 512],
                                     QT_sb[:, i * BL:(i + 1) * BL],
                                     wq_st[:, n * 512:(n + 1) * 512],
                                     start=(i == 0), stop=(i == 7))
            qh_sb = wpool.tile([BL, D], F32)
            nc.vector.tensor_add(qh_sb[:], qhp[:], bq8[:])
            qtp2 = spsum.tile([128, 8 * BL], F32, tag="qtp2")
            for m in range(8):
                nc.tensor.transpose(qtp2[:, m * BL:(m + 1) * BL],
                                    qh_sb[:, m * 128:(m + 1) * 128],
                                    ident[:BL, :BL])
            qhT_sb = wpool.tile([128, 8 * BL], F32)  # [p, m*BL + b]
            nc.vector.tensor_copy(qhT_sb[:], qtp2[:])

            # WkT via staged transposes
            WkT = [wpool.tile([128, D], F32R, tag=f"wkt{c}", name=f"wkt{c}")
                   for c in range(8)]
            for a in range(8):
                wk_st = wstage.tile([128, D], F32, tag="wk_st", name="wk_st")
                nc.sync.dma_start(wk_st[:], Wk_ext[a * 128:(a + 1) * 128, :])
                wp = spsum.tile([128, D], F32, tag="wtp", name="wp")
                for c in range(8):
                    nc.tensor.transpose(wp[:, c * 128:(c + 1) * 128],
                                        wk_st[:, c * 128:(c + 1) * 128],
                                        ident[:])
                for c in range(8):
                    nc.vector.tensor_copy(WkT[c][:, a * 128:(a + 1) * 128],
                                          wp[:, c * 128:(c + 1) * 128])

            # Block-diagonal qh for ALL batches:
            # qblk_c[p, b*16+h] = qh_b[c*128+p] if h == head(c*128+p) else 0
            qblk = [wpool.tile([128, 16 * BL], F32R, tag=f"qblk{c}", name=f"qblk{c}")
                    for c in range(8)]
            for c in range(8):
                nc.vector.tensor_copy(qblk[c][:], zeros32[:, :16 * BL])
                lo = qblk[c][0:64, :].rearrange("p (b h) -> p b h", h=H)
                hi = qblk[c][64:128, :].rearrange("p (b h) -> p b h", h=H)
                nc.vector.tensor_copy(
                    lo[:, :, 2 * c:2 * c + 1],
                    qhT_sb[0:64, c * BL:(c + 1) * BL].unsqueeze(2))
                nc.vector.tensor_copy(
                    hi[:, :, 2 * c + 1:2 * c + 2],
                    qhT_sb[64:128, c * BL:(c + 1) * BL].unsqueeze(2))
            RT_sb = wpool.tile([16 * BL, D], F32)  # [b*16+h, d]
            for n in range(2):
                rtp = spsum.tile([16 * BL, 512], F32, tag="rtp", name="rtp")
                for c in range(8):
                    nc.tensor.matmul(rtp[:], qblk[c][:],
                                     WkT[c][:, n * 512:(n + 1) * 512],
                                     start=(c == 0), stop=(c == 7))
                nc.vector.tensor_copy(RT_sb[:, n * 512:(n + 1) * 512], rtp[:])
            for j in range(8):
                rp = spsum.tile([128, 16 * BL], F32, tag="rp", name="rp")
                nc.tensor.transpose(rp[:], RT_sb[:, j * 128:(j + 1) * 128],
                                    ident[:16 * BL, :16 * BL])
                nc.vector.tensor_copy(R_all[:, j, :], rp[:])

        # tail weights live in their own pool, created after setup frees SBUF
        tailw = ctx.enter_context(tc.tile_pool(name="tailw", bufs=1))
        Wv_sb = [tailw.tile([128, D], F32, tag=f"wv{j}", name=f"wv{j}") for j in range(8)]
        Wo_r = [tailw.tile([128, D], BF16, tag=f"wor{j}", name=f"wor{j}") for j in range(8)]

        # ---------------- stream phase ----------------
        stream_psum = ExitStack()
        tpp = stream_psum.enter_context(tc.tile_pool(name="tpp", bufs=1, space="PSUM"))
        ktp = stream_psum.enter_context(tc.tile_pool(name="ktp", bufs=3, space="PSUM"))
        scp = stream_psum.enter_context(tc.tile_pool(name="scp", bufs=2, space="PSUM"))
        upp = stream_psum.enter_context(tc.tile_pool(name="upp", bufs=1, space="PSUM"))

        for b in range(BL):
            E_b = epool.tile([H, S], F32, tag="E")
            den4 = epool.tile([H, SG], F32, tag="den4")
            v_tiles = v_tiles0 if b == 0 else load_v(b)
            if b + 1 < BL:
                k_next = load_k(b + 1)
            if b == BL - 3:
                # prefetch tail weights so they arrive before the tail phase
                for j in range(8):
                    nc.sync.dma_start(Wv_sb[j][:], Wv_ext[j * 128:(j + 1) * 128, :])
                    nc.gpsimd.dma_start(Wo_r[j][:], Wo_ext[j * 128:(j + 1) * 128, :])
            for g in range(SG):
                kbf = k_tiles[g]
                kt4 = ktpool.tile([128, 8, 512], BF16, tag="kt4")
                for j in range(4):
                    for half in range(2):
                        tp = ktp.tile([128, 512], BF16, tag="ktp", name="tp")
                        for d4 in range(4):
                            dj = half * 4 + d4
                            nc.tensor.transpose(tp[:, d4 * 128:(d4 + 1) * 128],
                                                kbf[:, j, dj * 128:(dj + 1) * 128],
                                                ident_bf[:])
                        nc.vector.tensor_copy(
                            kt4[:, half * 4:(half + 1) * 4, j * 128:(j + 1) * 128],
                            tp[:].rearrange("p (a b) -> p a b", a=4))
                sc = scp.tile([H, 512], F32, tag="sc")
                for j in range(8):
                    nc.tensor.matmul(sc[:], R_all[:, j, b * H:(b + 1) * H],
                                     kt4[:, j, :],
                                     start=(j == 0), stop=(j == 7))
                nc.scalar.activation(E_b[:, g * 512:(g + 1) * 512], sc[:],
                                     ACTF.Exp, scale=0.125,
                                     accum_out=den4[:, g:g + 1])

            den = epool.tile([H, 1], F32, tag="den")
            nc.vector.tensor_reduce(den[:], den4[:], axis=AX.X, op=ALU.add)
            rden = epool.tile([H, 1], F32, tag="rden")
            nc.vector.reciprocal(rden[:], den[:])

            ET_b = etpool.tile([128, SC, H], BF16, tag="ET")
            gsz = min(8, SC)
            for tg in range(SC // gsz):
                sp = tpp.tile([128, gsz * H], F32, tag="tp", name="sp")
                for i in range(gsz):
                    t = tg * gsz + i
                    nc.tensor.transpose(sp[:, i * H:(i + 1) * H],
                                        E_b[:, t * 128:(t + 1) * 128],
                                        ident[:H, :H])
                nc.vector.tensor_copy(
                    ET_b[:, tg * gsz:(tg + 1) * gsz, :],
                    sp[:, :gsz * H].rearrange("p (t h) -> p t h", t=gsz))

            up = upp.tile([H, D], F32, tag="up")
            for g in range(SG):
                vr = v_tiles[g]
                for j in range(4):
                    t = g * 4 + j
                    for n in range(2):
                        nc.tensor.matmul(up[:, n * 512:(n + 1) * 512],
                                         ET_b[:, t, :],
                                         vr[:, j, n * 512:(n + 1) * 512],
                                         start=(t == 0), stop=(t == SC - 1))
            U_sb = upool.tile([H, D], F32, tag="U")
            nc.vector.tensor_scalar_mul(U_sb[:], up[:], rden[:])

            sp = tpp.tile([128, 8 * H], F32, tag="tp")
            for jc in range(8):
                nc.tensor.transpose(sp[:, jc * H:(jc + 1) * H],
                                    U_sb[:, jc * 128:(jc + 1) * 128],
                                    ident[:H, :H])
            nc.vector.tensor_copy(
                UT_all[:, :, :, b],
                sp[:].rearrange("p (j h) -> p j h", j=8))
            if b + 1 < BL:
                k_tiles = k_next

        # ---------------- tail: out-projection ----------------
        stream_psum.close()
        with tc.tile_pool(name="fin", bufs=1) as fpool, \
             tc.tile_pool(name="fpsum", bufs=1, space="PSUM") as fpsum:
            oc = fpsum.tile([BL, D], F32, tag="oc")
            for h in range(H):
                for jc in range(8):
                    nc.tensor.matmul(oc[:, h * 64:(h + 1) * 64],
                                     UT_all[:, jc, h, :],
                                     Wv_sb[jc][:, h * 64:(h + 1) * 64],
                                     start=(jc == 0), stop=(jc == 7))
            OC_sb = fpool.tile([BL, D], F32)
            nc.vector.tensor_add(OC_sb[:], oc[:], bv8[:])

            op = fpsum.tile([128, 8 * BL], F32, tag="op")
            for jc in range(8):
                nc.tensor.transpose(op[:, jc * BL:(jc + 1) * BL],
                                    OC_sb[:, jc * 128:(jc + 1) * 128],
                                    ident[:BL, :BL])
            OCT = fpool.tile([128, 8, BL], BF16)
            nc.vector.tensor_copy(OCT[:], op[:].rearrange("p (j b) -> p j b", j=8))

            yp = fpsum.tile([BL, D], F32, tag="yp")
            for n in range(2):
                for jc in range(8):
                    nc.tensor.matmul(yp[:, n * 512:(n + 1) * 512],
                                     OCT[:, jc, :],
                                     Wo_r[jc][:, n * 512:(n + 1) * 512],
                                     start=(jc == 0), stop=(jc == 7))
            ytmp = fpool.tile([BL, D], F32)
            nc.vector.tensor_add(ytmp[:], yp[:], bo8[:])
            y_sb = fpool.tile([BL, D], F32)
            nc.vector.tensor_scalar_max(y_sb[:], ytmp[:], 0.0)
            nc.sync.dma_start(y_ext[:], y_sb[:])

    nc.compile()
    return nc


_built = {}


def _get_nc(BL, S):
    key = (BL, S)
    if key not in _built:
        _built[key] = build(BL, S)
    return _built[key]


def kernel(q, k, v, Wq, bq, Wk, bk, Wv, bv, Wo, bo, _trace=False):
    q = np.asarray(q, dtype=np.float32)
    k = np.asarray(k, dtype=np.float32)
    v = np.asarray(v, dtype=np.float32)
    B, S = k.shape[0], k.shape[1]
    BL = B // N_CORES
    nc = _get_nc(BL, S)

    shared = {
        "Wq": np.ascontiguousarray(Wq, dtype=np.float32),
        "Wk": np.ascontiguousarray(Wk, dtype=np.float32),
        "Wv": np.ascontiguousarray(Wv, dtype=np.float32),
        "Wo": np.ascontiguousarray(Wo, dtype=np.float32),
        "bq": np.ascontiguousarray(bq, dtype=np.float32),
        "bv": np.ascontiguousarray(bv, dtype=np.float32),
        "bo": np.ascontiguousarray(bo, dtype=np.float32),
    }
    in_maps = []
    for c in range(N_CORES):
        sl = slice(c * BL, (c + 1) * BL)
        in_maps.append({
            "q": np.ascontiguousarray(q[sl].reshape(BL, D)),
            "k": np.ascontiguousarray(k[sl].reshape(BL * S, D)),
            "v": np.ascontiguousarray(v[sl].reshape(BL * S, D)),
            **shared,
        })
    res = run_bass_kernel_spmd(nc, in_maps, list(range(N_CORES)), trace=_trace)
    out = np.concatenate([res.results[c]["y"] for c in range(N_CORES)], axis=0)
    if _trace:
        kernel._last_exec_time_ns = res.exec_time_ns
        kernel._last_profile = res.profile_json
    return out

